# revision 9
# baseline (speedup 1.0000x reference)
import sys

if "/opt/trn_rl_repo" not in sys.path:
    sys.path.insert(0, "/opt/trn_rl_repo")

import numpy as np

import concourse.bass as bass
import concourse.tile as tile
from concourse import bacc, mybir

H = 2048
NCORES = 8
SL = H // NCORES  # 256 hidden dims per core
NKC = 16  # k-chunks of 128 over H
NT = 2  # gate groups per core -> psum partitions {0, 32}
GW = SL // NT  # 128 hidden dims per group
# free-col sections per group: [i | f | o | g] x GW
# pytorch row bases for (i, f, o, g):
SEC_BASE = [0, H, 3 * H, 2 * H]
F32 = mybir.dt.float32
_TRACE = {"on": False, "exec_ns": None}


def _build_nc(nsteps_dev, debug=False):
    """Device program: nsteps_dev LSTM steps, tensor-parallel over 8 cores.

    Per core: resident fused-W slice [128, NT, NKC+1, 4*GW]; gathered h in
    SBUF as [128, NKC+1] (col kk<16 holds h[16p+kk], col 16 = e0 bias
    selector). Each step: 2*(NKC+1) matmuls accumulate gates into PSUM
    partitions {0,32} x 512 cols, ACT/DVE elementwise, then AllGather of the
    new 256-wide h slice via collective (skipped on the last step).
    """
    nc = bacc.Bacc(target_bir_lowering=False, debug=debug)
    w_d = nc.declare_dram_parameter("wmov", [128, NT, NKC + 1, 4 * GW], F32, False)
    hinit_d = nc.declare_dram_parameter("hinit", [128, NKC + 1], F32, False)
    cinit_d = nc.declare_dram_parameter("cinit", [NT, GW], F32, False)
    hout_d = nc.declare_dram_parameter("hout", [NT, GW], F32, True)
    cout_d = nc.declare_dram_parameter("cout", [NT, GW], F32, True)

    with tile.TileContext(nc) as tc:
        with (
            tc.tile_pool(name="wpool", bufs=1) as wpool,
            tc.tile_pool(name="hgpool", bufs=2) as hgpool,
            tc.tile_pool(name="cpool", bufs=2) as cpool,
            tc.tile_pool(name="sgpool", bufs=2) as sgpool,
            tc.tile_pool(name="ewpool", bufs=8) as ewpool,
            tc.tile_pool(name="psum", bufs=2, space="PSUM") as psum,
            tc.tile_pool(name="dram", bufs=2, space="DRAM") as dram,
        ):
            w_s = wpool.tile([128, NT, NKC + 1, 4 * GW], F32)
            nc.sync.dma_start(w_s[:], w_d[:])

            hg = [
                hgpool.tile([128, NKC + 1], F32, name=f"hg{i}") for i in range(2)
            ]
            nc.sync.dma_start(hg[0][:], hinit_d[:])
            # second buffer only needs the constant e0 column
            nc.sync.dma_start(hg[1][:, NKC : NKC + 1], hinit_d[:, NKC : NKC + 1])

            c_cur = cpool.tile([128, GW], F32)
            nc.sync.dma_start(c_cur[0:64:32, :], cinit_d[:])

            hs_dram = dram.tile([SL], F32)
            hfull_dram = dram.tile([H], F32)

            h_ew = None
            for s in range(nsteps_dev):
                cur = hg[s % 2]
                nxt = hg[(s + 1) % 2]

                ps = psum.tile([128, 4 * GW], F32)
                for kk in range(NKC + 1):
                    for t in range(NT):
                        nc.tensor.matmul(
                            ps[32 * t : 32 * t + 1, :],
                            cur[:, kk : kk + 1],
                            w_s[:, t, kk, :],
                            start=(kk == 0),
                            stop=(kk == NKC),
                        )

                # HW compute engines need dense partition APs: per-group
                # single-partition ops at p in {0, 32}, interleaved so ACT
                # and DVE queues pipeline across the two groups.
                sg = sgpool.tile([128, 4 * GW], F32)
                fc = ewpool.tile([128, GW], F32)
                ig = ewpool.tile([128, GW], F32)
                c_new = cpool.tile([128, GW], F32)
                th = ewpool.tile([128, GW], F32)
                h_ew = ewpool.tile([128, GW], F32)
                P = [slice(32 * t, 32 * t + 1) for t in range(NT)]
                for t in range(NT):
                    nc.scalar.activation(
                        sg[P[t], 0 : 3 * GW],
                        ps[P[t], 0 : 3 * GW],
                        mybir.ActivationFunctionType.Sigmoid,
                    )
                for t in range(NT):
                    nc.scalar.activation(
                        sg[P[t], 3 * GW : 4 * GW],
                        ps[P[t], 3 * GW : 4 * GW],
                        mybir.ActivationFunctionType.Tanh,
                    )
                for t in range(NT):
                    nc.vector.tensor_mul(
                        fc[P[t], :], sg[P[t], GW : 2 * GW], c_cur[P[t], :]
                    )
                    nc.vector.tensor_mul(
                        ig[P[t], :], sg[P[t], 0:GW], sg[P[t], 3 * GW : 4 * GW]
                    )
                for t in range(NT):
                    nc.vector.tensor_add(c_new[P[t], :], fc[P[t], :], ig[P[t], :])
                for t in range(NT):
                    nc.scalar.activation(
                        th[P[t], :],
                        c_new[P[t], :],
                        mybir.ActivationFunctionType.Tanh,
                    )
                for t in range(NT):
                    nc.vector.tensor_mul(
                        h_ew[P[t], :], sg[P[t], 2 * GW : 3 * GW], th[P[t], :]
                    )
                c_cur = c_new

                if s < nsteps_dev - 1:
                    nc.sync.dma_start(hs_dram[:], h_ew[0:64:32, :])
                    nc.gpsimd.collective_compute(
                        "AllGather",
                        mybir.AluOpType.bypass,
                        replica_groups=[list(range(NCORES))],
                        ins=[hs_dram.opt()],
                        outs=[hfull_dram.opt()],
                    )
                    nc.sync.dma_start(nxt[:, 0:NKC], hfull_dram[:])

            nc.sync.dma_start(hout_d[:], h_ew[0:64:32, :])
            nc.sync.dma_start(cout_d[:], c_cur[0:64:32, :])

    return nc


def _sigmoid(x):
    return 1.0 / (1.0 + np.exp(-x))


def _host_step1(x0, W_ih, b):
    g = W_ih @ x0 + b  # [4H]
    i = _sigmoid(g[0:H])
    f = _sigmoid(g[H : 2 * H])
    gg = np.tanh(g[2 * H : 3 * H])
    o = _sigmoid(g[3 * H : 4 * H])
    c1 = i * gg  # c0 = 0 so f*c0 drops
    h1 = o * np.tanh(c1)
    return h1.astype(np.float32), c1.astype(np.float32)


def _prep_in_maps(h1, c1, Wf, b):
    """Per-core inputs. wmov[p,t,kk,s*GW+col] = Wf[row, 16p+kk] (kk<16);
    kk=16 row-0 carries the bias."""
    in_maps = []
    hinit = np.zeros((128, NKC + 1), np.float32)
    hinit[:, 0:NKC] = h1.reshape(128, NKC)
    hinit[0, NKC] = 1.0
    for j in range(NCORES):
        wm = np.zeros((128, NT, NKC + 1, 4 * GW), np.float32)
        for t in range(NT):
            for sidx, base in enumerate(SEC_BASE):
                rows = base + SL * j + GW * t + np.arange(GW)
                sub = Wf[rows, :].reshape(GW, 128, NKC)  # [col, p, kk]
                wm[:, t, 0:NKC, sidx * GW : (sidx + 1) * GW] = sub.transpose(1, 2, 0)
                wm[0, t, NKC, sidx * GW : (sidx + 1) * GW] = b[rows]
        cinit = c1[SL * j : SL * (j + 1)].reshape(NT, GW).copy()
        in_maps.append({"wmov": wm, "hinit": hinit, "cinit": cinit})
    return in_maps


def kernel(inputs, W_ih, W_hh, b_ih, b_hh, W_out, b_out, steps):
    from concourse.bass_utils import run_bass_kernel_spmd

    inputs = np.asarray(inputs, np.float32)
    W_ih = np.asarray(W_ih, np.float32)
    W_hh = np.asarray(W_hh, np.float32)
    b = (np.asarray(b_ih, np.float32) + np.asarray(b_hh, np.float32)).astype(
        np.float32
    )
    W_out = np.asarray(W_out, np.float32)
    b_out = np.asarray(b_out, np.float32)
    nsteps = int(steps)
    assert nsteps == 512, nsteps

    h1, c1 = _host_step1(inputs[0], W_ih, b)
    Wf = (W_ih + W_hh).astype(np.float32)
    in_maps = _prep_in_maps(h1, c1, Wf, b)

    nc = _build_nc(nsteps - 1)
    nc.finalize()
    br = run_bass_kernel_spmd(
        nc, in_maps, list(range(NCORES)), trace=_TRACE["on"]
    )
    _TRACE["exec_ns"] = br.exec_time_ns
    res = br.results

    h_full = np.concatenate([res[j]["hout"].reshape(SL) for j in range(NCORES)])
    c_full = np.concatenate([res[j]["cout"].reshape(SL) for j in range(NCORES)])

    logits = W_out @ h_full + b_out
    m = logits.max()
    e = np.exp(logits - m)
    probs = (e / e.sum()).astype(np.float32)
    return (
        probs[None, :],
        h_full[None, :].astype(np.float32),
        c_full[None, :].astype(np.float32),
    )
